# revision 20
# baseline (speedup 1.0000x reference)
"""Trainium2 Bass kernel for blocked-DCT high-frequency extractor.

Computes, for x (64, 3, 512, 512) f32:
  gray = 0.299*R + 0.587*G + 0.114*B                     (B,1,H,W)
  per 8x8 block:  Y = mask * (D @ block @ D.T)           (2D DCT + high-pass)
  output (64, 1, 512, 512) f32

Strategy (pure data parallel over batch, 8 batches/core on 8 cores; the
kernel is HBM-bound: 24 MiB in + 8 MiB out per core, ~358 GB/s/core HBM
=> ~94 us floor; every compute engine is kept well under that wall).

Per core, per (batch, 128-row chunk) of the image:
  1. One 768 KB SWDGE DMA (gpsimd queue) brings all 3 channel chunks into
     a (128h, 3*512w) tile, CASTING fp32 -> bf16 in the DMA datapath.
  2. Grayscale in bf16, split DVE/ACT: g0 = x0*(w0/w2) + x2 (DVE STT),
     gs = x1*(w1/w2) (ACT), g1 = g0 + gs (ACT).
  3. H-direction DCT: one bf16 matmul.  The stationary weight is
     w2 * (I_16 kron D^T) with COLUMNS PERMUTED so the output row index
     is i' = u*16 + hb (u = h-frequency, hb = block row).  After the
     32x32 block transpose this puts u//2 into the partition-block index
     of the next matmul's output.
  4. ACT copies PSUM -> SBUF with fp32 -> bf16 cast (ISA forbids casts
     inside StreamTranspose), then DVE does the 32x32 block transpose.
  5. W-direction DCT: one bf16 matmul whose stationary weight is
     I_16 kron D^T with the high-pass mask FOLDED IN: with the permuted
     layout, mask==0 exactly on output partitions {l < 64 and l%8 < 4},
     so those weight columns are simply zeroed.  No mask stage at all.
  6. DVE 32x32 block transpose straight out of PSUM (fp32).
  7. 256 KB output DMA on the SP HWDGE queue whose DRAM access pattern
     un-permutes the rows: partition c2*32+c1*16+hb -> row hb*8+c2*2+c1.

The two matmuls are bf16 (the PE idles most of the kernel, so HAM holds
it at the cold 1.2 GHz clock; fp32 matmuls at 4x cycles were the
original bottleneck).  All intermediate precision is bf16, fine for the
2e-2 relative-error gate (measured ~6e-3).
"""

import os

import numpy as np

import concourse.bacc as bacc
import concourse.mybir as mybir
import concourse.tile as tile
from concourse.bass_utils import run_bass_kernel_spmd

N_CORES = 8
B, C, H, W = 64, 3, 512, 512
BLOC = B // N_CORES  # batches per core
P = 128              # SBUF partitions / chunk height
NCH = H // P         # 128-row chunks per image
F32 = mybir.dt.float32
BF16 = mybir.dt.bfloat16
GRAY_W = (0.299, 0.587, 0.114)

_NC = None          # cached compiled Bass module
LAST_RUN = None     # BassKernelResults of the most recent run (for test.py)


def _build_bass():
    nc = bacc.Bacc(
        "TRN2",
        target_bir_lowering=False,
        debug=False,
        num_devices=N_CORES,
    )
    x = nc.declare_dram_parameter("x", [BLOC, C, H, W], F32, isOutput=False)
    wts1 = nc.declare_dram_parameter("wts1", [C, P, P], BF16, isOutput=False)
    wts2 = nc.declare_dram_parameter("wts2", [1, P, P], BF16, isOutput=False)
    out = nc.declare_dram_parameter("out", [BLOC, 1, H, W], F32, isOutput=True)

    with tile.TileContext(nc) as tc:
        with (
            tc.tile_pool(name="consts", bufs=1) as consts,
            tc.tile_pool(name="xin", bufs=10) as xin,
            tc.tile_pool(name="work", bufs=8) as work,
            tc.tile_pool(name="psum", bufs=4, space="PSUM") as psum_pool,
        ):
            w1 = consts.tile([P, C * P], BF16, tag="w1")
            nc.sync.dma_start(
                w1[:].rearrange("p (c q) -> p c q", q=P),
                wts1.rearrange("c p q -> p c q"),
            )
            w2 = consts.tile([P, P], BF16, tag="w2")
            nc.sync.dma_start(w2[:], wts2[0])

            iters = []
            for b in range(BLOC):
                for hc in range(NCH):
                    iters.append((b, hc))

            # Software-pipelined by one iteration: MM2(i-1) is issued
            # between MM1-group(i) and the cast/transpose of iteration i,
            # so the in-order PE queue never parks waiting for the
            # ACT-cast + DVE-transpose round trip of its own iteration.
            def tail(pend):
                s1t_p, bp, hcp = pend
                # W-direction DCT, high-pass mask folded into wts2
                p2 = psum_pool.tile([P, W], F32, tag="p2")
                nc.tensor.matmul(p2[:], w2[:], s1t_p[:], start=True, stop=True)
                # block transpose back, straight out of PSUM
                s2t = work.tile([P, W], F32, tag="s2t", bufs=4)
                nc.vector.transpose(s2t[:], p2[:])
                # output DMA un-permutes the rows via the DRAM AP:
                # partition c2*32 + c1*16 + hb  ->  row hb*8 + c2*2 + c1
                dst = out[bp, 0, hcp * P:(hcp + 1) * P, :].rearrange(
                    "(hb c2 c1) w -> c2 c1 hb w", c2=4, c1=2
                )
                nc.sync.dma_start(dst, s2t[:])

            pending = None
            for b, hc in iters:
                # one 768 KB read: channels side by side in the free
                # dim, fp32 -> bf16 cast inline in the DMA (SWDGE)
                xt = xin.tile([P, C * W], BF16, tag="x")
                xsrc = x[b].rearrange("c (n p) w -> n p c w", p=P)[hc]
                nc.gpsimd.dma_start(
                    xt[:].rearrange("p (c w) -> p c w", w=W), xsrc
                )
                # H-direction DCT with the grayscale sum folded in as a
                # 3-matmul PSUM accumulation (permuted output rows):
                # p1 = sum_c (w_c * W1).T @ x_c
                p1 = psum_pool.tile([P, W], F32, tag="p1")
                for c in range(C):
                    nc.tensor.matmul(
                        p1[:], w1[:, c * P:(c + 1) * P],
                        xt[:, c * W:(c + 1) * W],
                        start=(c == 0), stop=(c == C - 1),
                    )
                # previous iteration's W-DCT + transpose + store
                if pending is not None:
                    tail(pending)
                # PSUM -> SBUF with fp32 -> bf16 cast, then 32x32 block
                # transpose -- both on DVE, keeping the iteration's
                # cross-engine chain confined to PE <-> DVE (+Sync store)
                s1 = work.tile([P, W], BF16, tag="s1", bufs=3)
                nc.vector.tensor_copy(s1[:], p1[:])
                s1t = work.tile([P, W], BF16, tag="s1t", bufs=3)
                nc.vector.transpose(s1t[:], s1[:])
                pending = (s1t, b, hc)
            tail(pending)
    nc.compile()
    return nc


def _host_constants(dct_matrix, mask):
    import ml_dtypes
    D = np.asarray(dct_matrix, dtype=np.float32)
    M = np.asarray(mask, dtype=np.float32)
    Wk = np.kron(np.eye(P // 8, dtype=np.float32), D.T).astype(np.float32)
    # MM1 stationary weights: per-channel grayscale-scaled, columns
    # permuted to i' = u*16 + hb
    perm = np.array([(i % 16) * 8 + (i // 16) for i in range(P)])
    w1 = np.stack(
        [np.float32(w) * Wk[:, perm] for w in GRAY_W]
    ).astype(ml_dtypes.bfloat16)
    # MM2 stationary: high-pass mask folded in.  In the permuted layout,
    # output partition l carries (u = 2*(l//32) + {0,1}, v = l%8); the
    # masked (u<4 & v<4) region is exactly {l < 64 and l%8 < 4}.
    # Use the actual mask values so any non-binary mask still works.
    colmask = np.empty(P, dtype=np.float32)
    for l in range(P):
        u = 2 * (l // 32)          # mask[:cutoff] rows are constant per pair
        colmask[l] = M[u, l % 8]
    w2 = (Wk * colmask[None, :]).astype(ml_dtypes.bfloat16)
    return w1, w2


def kernel(x, dct_matrix, mask):
    global _NC, LAST_RUN
    x = np.ascontiguousarray(np.asarray(x, dtype=np.float32))
    assert x.shape == (B, C, H, W)
    w1, w2 = _host_constants(dct_matrix, mask)

    if _NC is None:
        _NC = _build_bass()

    in_maps = [
        {"x": np.ascontiguousarray(x[i * BLOC:(i + 1) * BLOC]),
         "wts1": w1, "wts2": w2[None]}
        for i in range(N_CORES)
    ]
    trace = bool(int(os.environ.get("DCT_TRACE", "0")))
    tdir = os.environ.get("DCT_TRACE_DIR")
    if tdir:
        os.makedirs(tdir, exist_ok=True)
    LAST_RUN = run_bass_kernel_spmd(
        _NC, in_maps, list(range(N_CORES)), trace=trace, tmpdir=tdir,
    )
    out = np.concatenate([LAST_RUN.results[i]["out"] for i in range(N_CORES)], axis=0)
    return out


# revision 22
# speedup vs baseline: 1.1953x; 1.1953x over previous
"""Trainium2 Bass kernel for blocked-DCT high-frequency extractor.

Computes, for x (64, 3, 512, 512) f32:
  gray = 0.299*R + 0.587*G + 0.114*B                     (B,1,H,W)
  per 8x8 block:  Y = mask * (D @ block @ D.T)           (2D DCT + high-pass)
  output (64, 1, 512, 512) f32

Strategy (pure data parallel over batch, 8 batches/core on 8 cores; the
kernel is HBM-bound: 24 MiB in + 8 MiB out per core, ~358 GB/s/core HBM
=> ~94 us floor; every compute engine is kept well under that wall).

Per core, per (batch, 128-row chunk) of the image:
  1. One 768 KB SWDGE DMA (gpsimd queue) brings all 3 channel chunks into
     a (128h, 3*512w) tile, CASTING fp32 -> bf16 in the DMA datapath.
  2. Grayscale in bf16, split DVE/ACT: g0 = x0*(w0/w2) + x2 (DVE STT),
     gs = x1*(w1/w2) (ACT), g1 = g0 + gs (ACT).
  3. H-direction DCT: one bf16 matmul.  The stationary weight is
     w2 * (I_16 kron D^T) with COLUMNS PERMUTED so the output row index
     is i' = u*16 + hb (u = h-frequency, hb = block row).  After the
     32x32 block transpose this puts u//2 into the partition-block index
     of the next matmul's output.
  4. ACT copies PSUM -> SBUF with fp32 -> bf16 cast (ISA forbids casts
     inside StreamTranspose), then DVE does the 32x32 block transpose.
  5. W-direction DCT: one bf16 matmul whose stationary weight is
     I_16 kron D^T with the high-pass mask FOLDED IN: with the permuted
     layout, mask==0 exactly on output partitions {l < 64 and l%8 < 4},
     so those weight columns are simply zeroed.  No mask stage at all.
  6. DVE 32x32 block transpose straight out of PSUM (fp32).
  7. 256 KB output DMA on the SP HWDGE queue whose DRAM access pattern
     un-permutes the rows: partition c2*32+c1*16+hb -> row hb*8+c2*2+c1.

The two matmuls are bf16 (the PE idles most of the kernel, so HAM holds
it at the cold 1.2 GHz clock; fp32 matmuls at 4x cycles were the
original bottleneck).  All intermediate precision is bf16, fine for the
2e-2 relative-error gate (measured ~6e-3).
"""

import os

import numpy as np

import concourse.bacc as bacc
import concourse.mybir as mybir
import concourse.tile as tile
from concourse.bass_utils import run_bass_kernel_spmd

N_CORES = 8
B, C, H, W = 64, 3, 512, 512
BLOC = B // N_CORES  # batches per core
P = 128              # SBUF partitions / chunk height
NCH = H // P         # 128-row chunks per image
F32 = mybir.dt.float32
BF16 = mybir.dt.bfloat16
GRAY_W = (0.299, 0.587, 0.114)

_NC = None          # cached compiled Bass module
LAST_RUN = None     # BassKernelResults of the most recent run (for test.py)


def _build_bass():
    nc = bacc.Bacc(
        "TRN2",
        target_bir_lowering=False,
        debug=False,
        num_devices=N_CORES,
    )
    x = nc.declare_dram_parameter("x", [BLOC, C, H, W], F32, isOutput=False)
    wts1 = nc.declare_dram_parameter("wts1", [C, P, P], BF16, isOutput=False)
    wts2 = nc.declare_dram_parameter("wts2", [1, P, P], BF16, isOutput=False)
    out = nc.declare_dram_parameter("out", [BLOC, 1, H, W], F32, isOutput=True)

    with tile.TileContext(nc) as tc:
        with (
            tc.tile_pool(name="consts", bufs=1) as consts,
            tc.tile_pool(name="xin", bufs=10) as xin,
            tc.tile_pool(name="work", bufs=8) as work,
            tc.tile_pool(name="psum", bufs=4, space="PSUM") as psum_pool,
        ):
            w1 = consts.tile([P, C * P], BF16, tag="w1")
            nc.sync.dma_start(
                w1[:].rearrange("p (c q) -> p c q", q=P),
                wts1.rearrange("c p q -> p c q"),
            )
            w2 = consts.tile([P, P], BF16, tag="w2")
            nc.sync.dma_start(w2[:], wts2[0])

            iters = []
            for b in range(BLOC):
                for hc in range(NCH):
                    iters.append((b, hc))

            # Software-pipelined by one iteration: MM2(i-1) is issued
            # between MM1-group(i) and the cast/transpose of iteration i,
            # so the in-order PE queue never parks waiting for the
            # ACT-cast + DVE-transpose round trip of its own iteration.
            def tail(pend):
                s1t_p, bp, hcp = pend
                # W-direction DCT, high-pass mask folded into wts2
                p2 = psum_pool.tile([P, W], F32, tag="p2")
                nc.tensor.matmul(p2[:], w2[:], s1t_p[:], start=True, stop=True)
                # block transpose back, straight out of PSUM
                s2t = work.tile([P, W], F32, tag="s2t", bufs=12)
                nc.vector.transpose(s2t[:], p2[:])
                # output DMA un-permutes the rows via the DRAM AP:
                # partition c2*32 + c1*16 + hb  ->  row hb*8 + c2*2 + c1
                dst = out[bp, 0, hcp * P:(hcp + 1) * P, :].rearrange(
                    "(hb c2 c1) w -> c2 c1 hb w", c2=4, c1=2
                )
                nc.sync.dma_start(dst, s2t[:])

            pending = None
            for b, hc in iters:
                # one 768 KB read: channels side by side in the free
                # dim, fp32 -> bf16 cast inline in the DMA (SWDGE)
                xt = xin.tile([P, C * W], BF16, tag="x")
                xsrc = x[b].rearrange("c (n p) w -> n p c w", p=P)[hc]
                nc.gpsimd.dma_start(
                    xt[:].rearrange("p (c w) -> p c w", w=W), xsrc
                )
                # H-direction DCT with the grayscale sum folded in as a
                # 3-matmul PSUM accumulation (permuted output rows):
                # p1 = sum_c (w_c * W1).T @ x_c
                p1 = psum_pool.tile([P, W], F32, tag="p1")
                for c in range(C):
                    nc.tensor.matmul(
                        p1[:], w1[:, c * P:(c + 1) * P],
                        xt[:, c * W:(c + 1) * W],
                        start=(c == 0), stop=(c == C - 1),
                    )
                # previous iteration's W-DCT + transpose + store
                if pending is not None:
                    tail(pending)
                # PSUM -> SBUF with fp32 -> bf16 cast, then 32x32 block
                # transpose -- both on DVE, keeping the iteration's
                # cross-engine chain confined to PE <-> DVE (+Sync store)
                s1 = work.tile([P, W], BF16, tag="s1")
                nc.vector.tensor_copy(s1[:], p1[:])
                s1t = work.tile([P, W], BF16, tag="s1t")
                nc.vector.transpose(s1t[:], s1[:])
                pending = (s1t, b, hc)
            tail(pending)
    nc.compile()
    return nc


def _host_constants(dct_matrix, mask):
    import ml_dtypes
    D = np.asarray(dct_matrix, dtype=np.float32)
    M = np.asarray(mask, dtype=np.float32)
    Wk = np.kron(np.eye(P // 8, dtype=np.float32), D.T).astype(np.float32)
    # MM1 stationary weights: per-channel grayscale-scaled, columns
    # permuted to i' = u*16 + hb
    perm = np.array([(i % 16) * 8 + (i // 16) for i in range(P)])
    w1 = np.stack(
        [np.float32(w) * Wk[:, perm] for w in GRAY_W]
    ).astype(ml_dtypes.bfloat16)
    # MM2 stationary: high-pass mask folded in.  In the permuted layout,
    # output partition l carries (u = 2*(l//32) + {0,1}, v = l%8); the
    # masked (u<4 & v<4) region is exactly {l < 64 and l%8 < 4}.
    # Use the actual mask values so any non-binary mask still works.
    colmask = np.empty(P, dtype=np.float32)
    for l in range(P):
        u = 2 * (l // 32)          # mask[:cutoff] rows are constant per pair
        colmask[l] = M[u, l % 8]
    w2 = (Wk * colmask[None, :]).astype(ml_dtypes.bfloat16)
    return w1, w2


def kernel(x, dct_matrix, mask):
    global _NC, LAST_RUN
    x = np.ascontiguousarray(np.asarray(x, dtype=np.float32))
    assert x.shape == (B, C, H, W)
    w1, w2 = _host_constants(dct_matrix, mask)

    if _NC is None:
        _NC = _build_bass()

    in_maps = [
        {"x": np.ascontiguousarray(x[i * BLOC:(i + 1) * BLOC]),
         "wts1": w1, "wts2": w2[None]}
        for i in range(N_CORES)
    ]
    trace = bool(int(os.environ.get("DCT_TRACE", "0")))
    tdir = os.environ.get("DCT_TRACE_DIR")
    if tdir:
        os.makedirs(tdir, exist_ok=True)
    LAST_RUN = run_bass_kernel_spmd(
        _NC, in_maps, list(range(N_CORES)), trace=trace, tmpdir=tdir,
    )
    out = np.concatenate([LAST_RUN.results[i]["out"] for i in range(N_CORES)], axis=0)
    return out
